# revision 17
# baseline (speedup 1.0000x reference)
"""Trainium2 Bass kernel for nn_MlroleNode_64716567216639 (GAT message passing).

Math note: the reference computes a dense NxN GATv2 attention but only row 0
of the output feeds the final MLP, so this kernel computes just that row:
e[j,h] = leaky(g_l[j] + g_r[0]) . w_attn over the 1024 source nodes, softmax,
weighted sum of g_r values, then the 3-layer type-define MLP over the 1023
ambiguous nodes (sharded 128 nodes per core; GAT row-0 replicated).

Optimizations vs the naive version:
- All inputs packed into ONE bf16 blob + one tiny fp32 blob -> 3 dma_starts
  instead of 22 (each dma_start costs ~600ns serially on the Sync engine).
- 64-row weights ride the unused bottom partitions (64:128) of the blob; the
  matmuls that consume them run in the lower PE quadrant via
  tile_position=(64, .).
- bf16 matmuls: single PE pass (fp32 runs LOW_HIGH = 4 passes).
- leaky(x + bias) fused into one scalar-engine ACT (Prelu, alpha=0.2) reading
  straight from PSUM. Prelu lives in the same ACT table as Exp -> no table
  switches; Sigmoid's table is preloaded via a dummy ACT after the last Exp.
- softmax 1/sum via the single-op approximate reciprocal instead of the
  ~1.1us DVE reciprocal.
"""
import numpy as np

H = 64
N_AMB = 1023
N = 1024
HEADS = 4
RT = 4
APT = 3
SLOPE = 0.2
NCORES = 8
SHARD = 128

# bf16 blob column map (see _prep_inputs)
C_WL = 0        # top: W_l.T            [64, 256]
C_WR = 256      # top: W_r.T            [64, 256]
C_HT = 512      # top: hT (node 0 = h1 slot, zero), nodes j at col C_HT+j
C_WT = 0        # bottom: W_trans[t].T/3  [64, 256]
C_WSELF = 256   # bottom: W_self.T      [64, 64]
C_WML = 320     # bottom: W_merge[:, :64].T
C_WMR = 384     # bottom: W_merge[:, 64:].T
C_TA = 448      # bottom: type agents   [64, 12]
C_BTT = 460     # bottom: b_trans.T     [64, 4]
C_HID = 464     # bottom: hidden.T      [64, 1]
C_BSC = 465     # bottom: b_self        [64, 1]
C_BMC = 466     # bottom: b_merge       [64, 1]
C_WD0B = 468    # bottom: Wd0[:, 64:].T
C_MLP = 532     # bottom: per-core mlp amb slice [64, 128]
C_WD0A = 660    # bottom: Wd0[:, :64].T
C_WD1 = 724     # bottom: Wd1.T         [64, 128]
C_WEXP = 1536   # full: block-diag w_attn  [128, 128]
C_WD2 = 1664    # full: Wd2.T           [128, 4]
C_BD1 = 1668    # full col: bd1
C_BD0 = 1669    # top rows 0:64: bd0
C_BD2 = 1670    # top rows 0:4: bd2
CB = 1671

_compiled = None
_flags_patched = False


def _patch_walrus_flags():
    """Append --max-sem-num to the walrus codegen invocation. The NEFF
    epilogue clears every allocatable semaphore one instruction at a time
    (~115ns each, split across engines); capping the allocatable range at 40
    instead of 256 cuts ~5us of teardown from every execution."""
    global _flags_patched
    if _flags_patched:
        return
    _flags_patched = True
    from concourse import bass_utils

    orig = bass_utils.bir_verify_and_optimise

    def patched(tmpdir, inp="bir.json", outp="file.neff", arch=None, *,
                dve_root=None):
        orig_run = bass_utils.run_command

        def run2(cmd, cwd=None, **kw):
            if (isinstance(cmd, list) and "walrus_driver" in str(cmd[0])
                    and "codegen" in ",".join(map(str, cmd))):
                cmd = list(cmd) + ["--max-sem-num=40"]
            return orig_run(cmd, cwd=cwd, **kw)

        bass_utils.run_command = run2
        try:
            return orig(tmpdir, inp, outp, arch, dve_root=dve_root)
        finally:
            bass_utils.run_command = orig_run

    bass_utils.bir_verify_and_optimise = patched


def _build():
    _patch_walrus_flags()
    import concourse.tile as tile
    from concourse import bacc, mybir

    f32 = mybir.dt.float32
    bf = mybir.dt.bfloat16
    AF = mybir.ActivationFunctionType
    ALU = mybir.AluOpType
    AX = mybir.AxisListType

    nc = bacc.Bacc("TRN2", target_bir_lowering=False, debug=False,
                   enable_asserts=False, num_devices=NCORES)

    bfb_d = nc.dram_tensor("bfb", [128, CB], bf, kind="ExternalInput").ap()
    outT_d = nc.dram_tensor("outT", [RT, SHARD], f32, kind="ExternalOutput").ap()

    with nc.allow_low_precision("bf16 kernel, tolerance 2e-2"), \
         tile.TileContext(nc) as tc:
        with tc.tile_pool(name="wp", bufs=1) as wp, \
             tc.tile_pool(name="sb", bufs=1) as sb, \
             tc.tile_pool(name="ps", bufs=1, space="PSUM") as ps:

            B = wp.tile([128, CB], bf, tag="bfb")
            # ordered by when consumers need the data: prologue weights
            # first (the serial merge chain is the head of the critical
            # path), then biases, the full top half (W_l/W_r + hT), the
            # full-height tail (Wexp/fold/Wd2), and the MLP weights last.
            # Bottom cols 852:1536 are zeros and never transferred.
            nc.sync.dma_start(B[64:128, C_WSELF:C_WD0B], bfb_d[64:128, C_WSELF:C_WD0B])
            nc.sync.dma_start(B[64:128, 0:C_WSELF], bfb_d[64:128, 0:C_WSELF])
            nc.sync.dma_start(B[0:64, 0:C_WEXP], bfb_d[0:64, 0:C_WEXP])
            nc.sync.dma_start(B[:, C_WEXP:CB], bfb_d[:, C_WEXP:CB])
            nc.sync.dma_start(B[64:128, C_WD0B:852], bfb_d[64:128, C_WD0B:852])

            # biases travel as bf16 in the blob; DVE ops need fp32 scalar
            # operands, so widen them into small fp32 tiles right after the
            # carrying DMA lands (all off the critical path)
            biasP = sb.tile([128, 2], f32, tag="biasP")
            nc.vector.tensor_copy(biasP[64:128, :], B[64:128, C_BSC:C_BSC + 2])
            bsc = biasP[64:128, 0:1]
            bmc = biasP[64:128, 1:2]
            biasM = sb.tile([128, 3], f32, tag="biasM")
            nc.vector.tensor_copy(biasM[:, 0:1], B[0:128, C_BD1:C_BD1 + 1])
            nc.vector.tensor_copy(biasM[0:64, 1:3], B[0:64, C_BD0:C_BD0 + 2])
            bd1c = biasM[0:128, 0:1]
            bd0c = biasM[0:64, 1:2]
            bd2c = biasM[0:4, 2:3]

            # preload the Exp table off the critical path (Prelu/Identity/Exp
            # all live in the same table set)
            warm = wp.tile([1, 4], f32, tag="warm")
            nc.vector.memset(warm[:], 0.0)
            warm_act = wp.tile([1, 4], f32, tag="warmact")
            nc.scalar.activation(warm_act[0:1, 0:1], warm[0:1, 0:1], AF.Exp)

            # PSUM arenas for small matmul outputs (bank-granular alloc).
            # Two separate banks: the tile-level dependency tracking adds
            # false write-after-read ordering between unrelated regions of
            # one tile, which serialized the prologue when shared.
            arenaM = ps.tile([128, 512], f32, tag="spM", bufs=1)
            tmp_ps = arenaM[64:128, 392:396]
            C_ps = arenaM[64:128, 400:404]
            y0_ps = arenaM[0:64, 0:SHARD]
            y1_ps = arenaM[0:128, 128:256]
            o_ps = arenaM[0:4, 256:384]
            h2p_ps = arenaM[0:64, 384:386]
            c0_ps = arenaM[0:64, 388:389]
            h1_ps = ps.tile([128, 1], f32, tag="hp", bufs=2)

            # ---- prologue ----
            # h1 = W_self @ hidden + b_self goes first: it heads the serial
            # merge chain, and the C-path below runs concurrently with it
            h1t = sb.tile([128, RT + 1], bf, tag="h1t")
            nc.tensor.matmul(h1_ps[64:128, :], B[64:128, C_WSELF:C_WSELF + H],
                             B[64:128, C_HID:C_HID + 1], start=True, stop=True,
                             tile_position=(64, 64))
            nc.scalar.activation(h1t[64:128, 0:1], h1_ps[64:128, :], AF.Identity,
                                 bias=bsc)

            # role-type routing (the per-iteration bias columns C_sb)
            tsum = sb.tile([128, RT], bf, tag="tsum")
            nc.vector.reduce_sum(
                tsum[64:128, :],
                B[64:128, C_TA:C_TA + RT * APT].rearrange("p (t a) -> p t a", a=APT),
                axis=AX.X)
            for t in range(RT):
                nc.tensor.matmul(tmp_ps[:, t:t + 1],
                                 B[64:128, C_WT + H * t:C_WT + H * (t + 1)],
                                 tsum[64:128, t:t + 1], start=True, stop=True,
                                 tile_position=(64, 64))
            tmpc = sb.tile([128, RT], bf, tag="tmpc")
            nc.vector.tensor_tensor(tmpc[64:128, :], tmp_ps[:],
                                    B[64:128, C_BTT:C_BTT + RT], op=ALU.add)
            nc.tensor.matmul(C_ps[:], B[64:128, C_WMR:C_WMR + H],
                             tmpc[64:128, :], start=True, stop=True,
                             tile_position=(64, 64))
            C_sb = sb.tile([128, RT], f32, tag="C")
            nc.vector.tensor_scalar_add(C_sb[64:128, :], C_ps[:], bmc)

            # 4x leaky-merge chain
            for t in range(RT):
                hp = ps.tile([128, 1], f32, tag="hp", bufs=2)
                nc.tensor.matmul(hp[64:128, :], B[64:128, C_WML:C_WML + H],
                                 h1t[64:128, t:t + 1], start=True, stop=True,
                                 tile_position=(64, 64))
                if t < RT - 1:
                    nc.scalar.activation(h1t[64:128, t + 1:t + 2], hp[64:128, :],
                                         AF.Prelu, bias=C_sb[64:128, t:t + 1],
                                         alpha=SLOPE)
                else:
                    # final h1 -> node-0 column of hT (top half)
                    nc.scalar.activation(B[0:64, C_HT:C_HT + 1], hp[64:128, :],
                                         AF.Prelu, bias=C_sb[64:128, t:t + 1],
                                         alpha=SLOPE)

            # attention query columns g_r[0] per head-pair block
            gr0c = sb.tile([128, 2], f32, tag="gr0c")
            for b in range(2):
                gr0_ps = ps.tile([128, 1], f32, tag="gr0", bufs=2)
                nc.tensor.matmul(gr0_ps[:], B[0:64, C_WR + 128 * b:C_WR + 128 * b + 128],
                                 B[0:64, C_HT:C_HT + 1], start=True, stop=True)
                nc.vector.tensor_copy(gr0c[:, b:b + 1], gr0_ps[:])

            # duplicated hT chunks: top = bottom = chunk, so one DVE pass
            # per unit can weight BOTH heads (pexp rows 0:64 and 64:128)
            # against the node features with all operands at base partition 0
            hdup = wp.tile([128, N], bf, tag="hdup")
            for c in range(2):
                cols = slice(C_HT + 512 * c, C_HT + 512 * (c + 1))
                nc.vector.tensor_copy(hdup[0:64, 512 * c:512 * (c + 1)], B[0:64, cols])
                nc.vector.tensor_copy(hdup[64:128, 512 * c:512 * (c + 1)], B[0:64, cols])


            # ---- first MLP matmul on this core's shard (h2-independent) ----
            nc.tensor.matmul(y0_ps, B[64:128, C_WD0A:C_WD0A + H],
                             B[64:128, C_MLP:C_MLP + SHARD], start=True, stop=True,
                             tile_position=(64, 0))

            # ---- GAT row 0: 2 head-pair blocks x 2 column chunks of 512.
            # Value aggregation uses linearity: sum_j a_j (W_r h_j) =
            # W_r (sum_j a_j h_j), so no big g_r matmuls are needed; the
            # weighted sums run on DVE straight against the bf16 hT columns
            # and W_r is applied once per head to a single 64-vector. ----
            ssum4 = sb.tile([128, 4], f32, tag="ssum4")
            vparts = sb.tile([128, 4], f32, tag="vparts")  # col = unit
            # pass 1: gl matmuls + fused leaky(gl + gr0) -> t_sb
            gl_list, t_list = [], []
            for b in range(2):
                for c in range(2):
                    cols = slice(C_HT + 512 * c, C_HT + 512 * (c + 1))
                    gl_ps = ps.tile([128, 512], f32, tag="ge", bufs=3)
                    nc.tensor.matmul(gl_ps[:],
                                     B[0:64, C_WL + 128 * b:C_WL + 128 * b + 128],
                                     B[0:64, cols], start=True, stop=True)
                    t_sb = sb.tile([128, 512], bf, tag="t", bufs=4)
                    nc.scalar.activation(t_sb[:], gl_ps[:], AF.Prelu,
                                         bias=gr0c[:, b:b + 1], alpha=SLOPE)
                    t_list.append(t_sb)
            # pass 2: attention logits -> exp -> per-head weighted node sums
            for b in range(2):
                for c in range(2):
                    u = 2 * b + c
                    cols = slice(C_HT + 512 * c, C_HT + 512 * (c + 1))
                    e_ps = ps.tile([128, 512], f32, tag="ge", bufs=3)
                    nc.tensor.matmul(e_ps[:], B[:, C_WEXP:C_WEXP + 128],
                                     t_list[u][:], start=True, stop=True)
                    pexp = sb.tile([128, 512], bf, tag="pexp", bufs=3)
                    nc.scalar.activation(pexp[:], e_ps[:], AF.Exp, bias=0.0,
                                         accum_out=ssum4[:, u:u + 1])
                    scr = sb.tile([128, 512], bf, tag="scr", bufs=3)
                    nc.vector.scalar_tensor_tensor(
                        out=scr[:], in0=pexp[:], scalar=1.0,
                        in1=hdup[:, 512 * c:512 * (c + 1)],
                        op0=ALU.mult, op1=ALU.mult,
                        accum_out=vparts[:, u:u + 1])

            # preload the Sigmoid table while the MLP matmuls run. Reading a
            # row of ssum4 makes this depend on ALL four Exp accumulators, so
            # the table switch is ordered strictly after the last Exp (the
            # same table also holds Prelu, so later Prelu ACTs don't reload).
            warm_sig = wp.tile([1, 4], f32, tag="warmsig")
            nc.scalar.activation(warm_sig[0:1, 0:4], ssum4[0:1, 0:4], AF.Sigmoid)

            # combine chunks, normalize (0.25 head-mean folded into the
            # reciprocal), apply W_r per head, accumulate h2 in PSUM
            ssum2 = sb.tile([128, 2], f32, tag="ssum2")
            v2 = sb.tile([128, 2], f32, tag="v2")
            sview = ssum4[:].rearrange("p (b c) -> p b c", c=2)
            vview = vparts[:].rearrange("p (b c) -> p b c", c=2)
            nc.vector.tensor_tensor(ssum2[:], sview[:, :, 0], sview[:, :, 1],
                                    op=ALU.add)
            nc.vector.tensor_tensor(v2[:], vview[:, :, 0], vview[:, :, 1],
                                    op=ALU.add)
            rs2 = sb.tile([128, 2], f32, tag="rs2")
            nc.vector.reciprocal_approx_fast(rs2[:], ssum2[:])
            vn2 = sb.tile([128, 2], bf, tag="vn2")
            # vn = v * (1/sum) * 0.25 (head mean) in one fused DVE op
            nc.vector.scalar_tensor_tensor(out=vn2[:], in0=v2[:],
                                           scalar=1.0 / HEADS, in1=rs2[:],
                                           op0=ALU.mult, op1=ALU.mult)
            vnb = sb.tile([64, 2], bf, tag="vnb")
            nc.vector.tensor_copy(vnb[:], vn2[64:128, :])
            h2_ps = h2p_ps[:, 0:1]
            for h in range(RT):
                b, i = divmod(h, 2)
                rhs = vnb[:, b:b + 1] if i else vn2[0:64, b:b + 1]
                nc.tensor.matmul(h2_ps, B[0:64, C_WR + 64 * h:C_WR + 64 * h + 64],
                                 rhs, start=(h == 0), stop=(h == 3))
            h2b = sb.tile([128, 1], bf, tag="h2b")
            nc.vector.tensor_copy(h2b[64:128, :], h2_ps)

            # ---- final MLP (gated on h2) ----
            nc.tensor.matmul(c0_ps, B[64:128, C_WD0B:C_WD0B + H],
                             h2b[64:128, :], start=True, stop=True,
                             tile_position=(64, 0))
            c0col = sb.tile([64, 1], f32, tag="c0col")
            nc.vector.tensor_scalar_add(c0col[:], c0_ps, bd0c)
            y0b = sb.tile([128, SHARD], bf, tag="y0b")
            nc.scalar.activation(y0b[64:128, :], y0_ps, AF.Prelu, bias=c0col[:],
                                 alpha=SLOPE)
            nc.tensor.matmul(y1_ps, B[64:128, C_WD1:C_WD1 + SHARD],
                             y0b[64:128, :], start=True, stop=True,
                             tile_position=(64, 0))
            y1f = sb.tile([128, SHARD], bf, tag="y1f")
            nc.scalar.activation(y1f[:], y1_ps, AF.Prelu, bias=bd1c, alpha=SLOPE)
            nc.tensor.matmul(o_ps, B[:, C_WD2:C_WD2 + RT], y1f[:],
                             start=True, stop=True)
            o_sb = sb.tile([RT, SHARD], f32, tag="osb")
            nc.scalar.activation(o_sb[:], o_ps, AF.Sigmoid, bias=bd2c)
            nc.sync.dma_start(outT_d[:], o_sb[:])

    nc.compile()
    return nc


def _prep_inputs(inputs):
    import ml_dtypes
    bf16 = ml_dtypes.bfloat16
    f32 = np.float32

    hidden = np.asarray(inputs["hidden"], f32)
    ambiguous = np.asarray(inputs["ambiguous"], f32)
    type_agents = np.asarray(inputs["type_agents"], f32)
    W_self = np.asarray(inputs["W_self"], f32)
    b_self = np.asarray(inputs["b_self"], f32)
    W_merge = np.asarray(inputs["W_merge"], f32)
    b_merge = np.asarray(inputs["b_merge"], f32)
    W_trans = np.asarray(inputs["W_trans"], f32)
    b_trans = np.asarray(inputs["b_trans"], f32)
    W_l = np.asarray(inputs["W_l"], f32)
    W_r = np.asarray(inputs["W_r"], f32)
    w_attn = np.asarray(inputs["w_attn"], f32)
    Wd0 = np.asarray(inputs["Wd0"], f32)
    bd0 = np.asarray(inputs["bd0"], f32)
    Wd1 = np.asarray(inputs["Wd1"], f32)
    bd1 = np.asarray(inputs["bd1"], f32)
    Wd2 = np.asarray(inputs["Wd2"], f32)
    bd2 = np.asarray(inputs["bd2"], f32)

    base = np.zeros((128, CB), f32)
    top = base[0:64]
    bot = base[64:128]
    top[:, C_WL:C_WL + 256] = W_l.T
    top[:, C_WR:C_WR + 256] = W_r.T
    top[:, C_HT + 1:C_HT + N] = ambiguous.T
    bot[:, C_WT:C_WT + 256] = np.concatenate(
        [W_trans[t].T for t in range(RT)], axis=1) / APT
    bot[:, C_WSELF:C_WSELF + H] = W_self.T
    bot[:, C_WML:C_WML + H] = W_merge[:, :H].T
    bot[:, C_WMR:C_WMR + H] = W_merge[:, H:].T
    bot[:, C_WD0B:C_WD0B + H] = Wd0[:, H:].T
    bot[:, C_WD0A:C_WD0A + H] = Wd0[:, :H].T
    bot[:, C_WD1:C_WD1 + SHARD] = Wd1.T
    bot[:, C_TA:C_TA + RT * APT] = type_agents.reshape(RT * APT, H).T
    bot[:, C_BTT:C_BTT + RT] = b_trans.T
    bot[:, C_HID:C_HID + 1] = hidden.T
    bot[:, C_BSC] = b_self
    bot[:, C_BMC] = b_merge
    wexp = np.zeros((128, 128), f32)
    for hh in range(2):
        wexp[hh * 64:(hh + 1) * 64, hh * 64:(hh + 1) * 64] = w_attn[:, None]
    base[:, C_WEXP:C_WEXP + 128] = wexp
    base[:, C_WD2:C_WD2 + RT] = Wd2.T
    base[:, C_BD1] = bd1
    top[:, C_BD0] = bd0
    base[0:RT, C_BD2] = bd2

    amb_pad = np.zeros((H, NCORES * SHARD), f32)
    amb_pad[:, :N_AMB] = ambiguous.T
    in_maps = []
    for cidx in range(NCORES):
        blob = base.copy()
        blob[64:128, C_MLP:C_MLP + SHARD] = \
            amb_pad[:, cidx * SHARD:(cidx + 1) * SHARD]
        in_maps.append({"bfb": blob.astype(bf16)})
    return in_maps


def kernel(**inputs) -> np.ndarray:
    global _compiled
    if _compiled is None:
        _compiled = _build()
    nc = _compiled
    from concourse import bass_utils

    in_maps = _prep_inputs(inputs)
    res = bass_utils.run_bass_kernel_spmd(nc, in_maps, core_ids=list(range(NCORES)))
    out = np.empty((N_AMB, RT), np.float32)
    for cidx in range(NCORES):
        lo = cidx * SHARD
        hi = min(lo + SHARD, N_AMB)
        out[lo:hi, :] = res.results[cidx]["outT"][:, :hi - lo].T
    return out


# revision 18
# speedup vs baseline: 1.0204x; 1.0204x over previous
"""Trainium2 Bass kernel for nn_MlroleNode_64716567216639 (GAT message passing).

Math note: the reference computes a dense NxN GATv2 attention but only row 0
of the output feeds the final MLP, so this kernel computes just that row:
e[j,h] = leaky(g_l[j] + g_r[0]) . w_attn over the 1024 source nodes, softmax,
weighted sum of g_r values, then the 3-layer type-define MLP over the 1023
ambiguous nodes (sharded 128 nodes per core; GAT row-0 replicated).

Optimizations vs the naive version:
- All inputs packed into ONE bf16 blob + one tiny fp32 blob -> 3 dma_starts
  instead of 22 (each dma_start costs ~600ns serially on the Sync engine).
- 64-row weights ride the unused bottom partitions (64:128) of the blob; the
  matmuls that consume them run in the lower PE quadrant via
  tile_position=(64, .).
- bf16 matmuls: single PE pass (fp32 runs LOW_HIGH = 4 passes).
- leaky(x + bias) fused into one scalar-engine ACT (Prelu, alpha=0.2) reading
  straight from PSUM. Prelu lives in the same ACT table as Exp -> no table
  switches; Sigmoid's table is preloaded via a dummy ACT after the last Exp.
- softmax 1/sum via the single-op approximate reciprocal instead of the
  ~1.1us DVE reciprocal.
"""
import numpy as np

H = 64
N_AMB = 1023
N = 1024
HEADS = 4
RT = 4
APT = 3
SLOPE = 0.2
NCORES = 8
SHARD = 128

# bf16 blob column map (see _prep_inputs)
C_WL = 0        # top: W_l.T            [64, 256]
C_WR = 256      # top: W_r.T            [64, 256]
C_HT = 512      # top: hT (node 0 = h1 slot, zero), nodes j at col C_HT+j
C_WT = 0        # bottom: W_trans[t].T/3  [64, 256]
C_WSELF = 256   # bottom: W_self.T      [64, 64]
C_WML = 320     # bottom: W_merge[:, :64].T
C_WMR = 384     # bottom: W_merge[:, 64:].T
C_TA = 448      # bottom: type agents   [64, 12]
C_BTT = 460     # bottom: b_trans.T     [64, 4]
C_HID = 464     # bottom: hidden.T      [64, 1]
C_BSC = 465     # bottom: b_self        [64, 1]
C_BMC = 466     # bottom: b_merge       [64, 1]
C_WD0B = 468    # bottom: Wd0[:, 64:].T
C_MLP = 532     # bottom: per-core mlp amb slice [64, 128]
C_WD0A = 660    # bottom: Wd0[:, :64].T
C_WD1 = 724     # bottom: Wd1.T         [64, 128]
C_WEXP = 1536   # full: block-diag w_attn  [128, 128]
C_WD2 = 1664    # full: Wd2.T           [128, 4]
C_BD1 = 1668    # full col: bd1
C_BD0 = 1669    # top rows 0:64: bd0
C_BD2 = 1670    # top rows 0:4: bd2
CB = 1671

_compiled = None


def _build():
    import concourse.tile as tile
    from concourse import bacc, mybir

    f32 = mybir.dt.float32
    bf = mybir.dt.bfloat16
    AF = mybir.ActivationFunctionType
    ALU = mybir.AluOpType
    AX = mybir.AxisListType

    nc = bacc.Bacc("TRN2", target_bir_lowering=False, debug=False,
                   enable_asserts=False, num_devices=NCORES)

    bfb_d = nc.dram_tensor("bfb", [128, CB], bf, kind="ExternalInput").ap()
    outT_d = nc.dram_tensor("outT", [RT, SHARD], f32, kind="ExternalOutput").ap()

    with nc.allow_low_precision("bf16 kernel, tolerance 2e-2"), \
         tile.TileContext(nc) as tc:
        with tc.tile_pool(name="wp", bufs=1) as wp, \
             tc.tile_pool(name="sb", bufs=1) as sb, \
             tc.tile_pool(name="ps", bufs=1, space="PSUM") as ps:

            B = wp.tile([128, CB], bf, tag="bfb")
            # ordered by when consumers need the data: prologue weights
            # first (the serial merge chain is the head of the critical
            # path), then biases, the full top half (W_l/W_r + hT), the
            # full-height tail (Wexp/fold/Wd2), and the MLP weights last.
            # Bottom cols 852:1536 are zeros and never transferred.
            nc.sync.dma_start(B[64:128, C_WSELF:C_WD0B], bfb_d[64:128, C_WSELF:C_WD0B])
            nc.sync.dma_start(B[64:128, 0:C_WSELF], bfb_d[64:128, 0:C_WSELF])
            nc.sync.dma_start(B[0:64, 0:C_WEXP], bfb_d[0:64, 0:C_WEXP])
            nc.sync.dma_start(B[:, C_WEXP:CB], bfb_d[:, C_WEXP:CB])
            nc.sync.dma_start(B[64:128, C_WD0B:852], bfb_d[64:128, C_WD0B:852])

            # biases travel as bf16 in the blob; DVE ops need fp32 scalar
            # operands, so widen them into small fp32 tiles right after the
            # carrying DMA lands (all off the critical path)
            biasP = sb.tile([128, 2], f32, tag="biasP")
            nc.vector.tensor_copy(biasP[64:128, :], B[64:128, C_BSC:C_BSC + 2])
            bsc = biasP[64:128, 0:1]
            bmc = biasP[64:128, 1:2]
            biasM = sb.tile([128, 3], f32, tag="biasM")
            nc.vector.tensor_copy(biasM[:, 0:1], B[0:128, C_BD1:C_BD1 + 1])
            nc.vector.tensor_copy(biasM[0:64, 1:3], B[0:64, C_BD0:C_BD0 + 2])
            bd1c = biasM[0:128, 0:1]
            bd0c = biasM[0:64, 1:2]
            bd2c = biasM[0:4, 2:3]

            # preload the Exp table off the critical path (Prelu/Identity/Exp
            # all live in the same table set)
            warm = wp.tile([1, 4], f32, tag="warm")
            nc.vector.memset(warm[:], 0.0)
            warm_act = wp.tile([1, 4], f32, tag="warmact")
            nc.scalar.activation(warm_act[0:1, 0:1], warm[0:1, 0:1], AF.Exp)

            # PSUM arenas for small matmul outputs (bank-granular alloc).
            # Two separate banks: the tile-level dependency tracking adds
            # false write-after-read ordering between unrelated regions of
            # one tile, which serialized the prologue when shared.
            arenaM = ps.tile([128, 512], f32, tag="spM", bufs=1)
            tmp_ps = arenaM[64:128, 392:396]
            C_ps = arenaM[64:128, 400:404]
            y0_ps = arenaM[0:64, 0:SHARD]
            y1_ps = arenaM[0:128, 128:256]
            o_ps = arenaM[0:4, 256:384]
            h2p_ps = arenaM[0:64, 384:386]
            c0_ps = arenaM[0:64, 388:389]
            h1_ps = ps.tile([128, 1], f32, tag="hp", bufs=2)

            # ---- prologue ----
            # h1 = W_self @ hidden + b_self goes first: it heads the serial
            # merge chain, and the C-path below runs concurrently with it
            h1t = sb.tile([128, RT + 1], bf, tag="h1t")
            nc.tensor.matmul(h1_ps[64:128, :], B[64:128, C_WSELF:C_WSELF + H],
                             B[64:128, C_HID:C_HID + 1], start=True, stop=True,
                             tile_position=(64, 64))
            nc.scalar.activation(h1t[64:128, 0:1], h1_ps[64:128, :], AF.Identity,
                                 bias=bsc)

            # role-type routing (the per-iteration bias columns C_sb)
            tsum = sb.tile([128, RT], bf, tag="tsum")
            nc.vector.reduce_sum(
                tsum[64:128, :],
                B[64:128, C_TA:C_TA + RT * APT].rearrange("p (t a) -> p t a", a=APT),
                axis=AX.X)
            for t in range(RT):
                nc.tensor.matmul(tmp_ps[:, t:t + 1],
                                 B[64:128, C_WT + H * t:C_WT + H * (t + 1)],
                                 tsum[64:128, t:t + 1], start=True, stop=True,
                                 tile_position=(64, 64))
            tmpc = sb.tile([128, RT], bf, tag="tmpc")
            nc.vector.tensor_tensor(tmpc[64:128, :], tmp_ps[:],
                                    B[64:128, C_BTT:C_BTT + RT], op=ALU.add)
            nc.tensor.matmul(C_ps[:], B[64:128, C_WMR:C_WMR + H],
                             tmpc[64:128, :], start=True, stop=True,
                             tile_position=(64, 64))
            C_sb = sb.tile([128, RT], f32, tag="C")
            nc.vector.tensor_scalar_add(C_sb[64:128, :], C_ps[:], bmc)

            # 4x leaky-merge chain
            for t in range(RT):
                hp = ps.tile([128, 1], f32, tag="hp", bufs=2)
                nc.tensor.matmul(hp[64:128, :], B[64:128, C_WML:C_WML + H],
                                 h1t[64:128, t:t + 1], start=True, stop=True,
                                 tile_position=(64, 64))
                if t < RT - 1:
                    nc.scalar.activation(h1t[64:128, t + 1:t + 2], hp[64:128, :],
                                         AF.Prelu, bias=C_sb[64:128, t:t + 1],
                                         alpha=SLOPE)
                else:
                    # final h1 -> node-0 column of hT (top half)
                    nc.scalar.activation(B[0:64, C_HT:C_HT + 1], hp[64:128, :],
                                         AF.Prelu, bias=C_sb[64:128, t:t + 1],
                                         alpha=SLOPE)

            # attention query columns g_r[0] per head-pair block
            gr0c = sb.tile([128, 2], f32, tag="gr0c")
            gr0_ps = ps.tile([128, 2], f32, tag="gr0", bufs=1)
            for b in range(2):
                nc.tensor.matmul(gr0_ps[:, b:b + 1],
                                 B[0:64, C_WR + 128 * b:C_WR + 128 * b + 128],
                                 B[0:64, C_HT:C_HT + 1], start=True, stop=True)
            nc.vector.tensor_copy(gr0c[:], gr0_ps[:])

            # duplicated hT chunks: top = bottom = chunk, so one DVE pass
            # per unit can weight BOTH heads (pexp rows 0:64 and 64:128)
            # against the node features with all operands at base partition 0
            hdup = wp.tile([128, N], bf, tag="hdup")
            for c in range(2):
                cols = slice(C_HT + 512 * c, C_HT + 512 * (c + 1))
                nc.vector.tensor_copy(hdup[0:64, 512 * c:512 * (c + 1)], B[0:64, cols])
                nc.vector.tensor_copy(hdup[64:128, 512 * c:512 * (c + 1)], B[0:64, cols])


            # ---- first MLP matmul on this core's shard (h2-independent) ----
            nc.tensor.matmul(y0_ps, B[64:128, C_WD0A:C_WD0A + H],
                             B[64:128, C_MLP:C_MLP + SHARD], start=True, stop=True,
                             tile_position=(64, 0))

            # ---- GAT row 0: 2 head-pair blocks x 2 column chunks of 512.
            # Value aggregation uses linearity: sum_j a_j (W_r h_j) =
            # W_r (sum_j a_j h_j), so no big g_r matmuls are needed; the
            # weighted sums run on DVE straight against the bf16 hT columns
            # and W_r is applied once per head to a single 64-vector. ----
            ssum4 = sb.tile([128, 4], f32, tag="ssum4")
            vparts = sb.tile([128, 4], f32, tag="vparts")  # col = unit
            # pass 1: gl matmuls + fused leaky(gl + gr0) -> t_sb
            gl_list, t_list = [], []
            for b in range(2):
                for c in range(2):
                    cols = slice(C_HT + 512 * c, C_HT + 512 * (c + 1))
                    gl_ps = ps.tile([128, 512], f32, tag="ge", bufs=3)
                    nc.tensor.matmul(gl_ps[:],
                                     B[0:64, C_WL + 128 * b:C_WL + 128 * b + 128],
                                     B[0:64, cols], start=True, stop=True)
                    t_sb = sb.tile([128, 512], bf, tag="t", bufs=4)
                    nc.scalar.activation(t_sb[:], gl_ps[:], AF.Prelu,
                                         bias=gr0c[:, b:b + 1], alpha=SLOPE)
                    t_list.append(t_sb)
            # pass 2: attention logits -> exp -> per-head weighted node sums
            for b in range(2):
                for c in range(2):
                    u = 2 * b + c
                    cols = slice(C_HT + 512 * c, C_HT + 512 * (c + 1))
                    e_ps = ps.tile([128, 512], f32, tag="ge", bufs=3)
                    nc.tensor.matmul(e_ps[:], B[:, C_WEXP:C_WEXP + 128],
                                     t_list[u][:], start=True, stop=True)
                    pexp = sb.tile([128, 512], bf, tag="pexp", bufs=3)
                    nc.scalar.activation(pexp[:], e_ps[:], AF.Exp, bias=0.0,
                                         accum_out=ssum4[:, u:u + 1])
                    scr = sb.tile([128, 512], bf, tag="scr", bufs=3)
                    nc.vector.scalar_tensor_tensor(
                        out=scr[:], in0=pexp[:], scalar=1.0,
                        in1=hdup[:, 512 * c:512 * (c + 1)],
                        op0=ALU.mult, op1=ALU.mult,
                        accum_out=vparts[:, u:u + 1])

            # preload the Sigmoid table while the MLP matmuls run. Reading a
            # row of ssum4 makes this depend on ALL four Exp accumulators, so
            # the table switch is ordered strictly after the last Exp (the
            # same table also holds Prelu, so later Prelu ACTs don't reload).
            warm_sig = wp.tile([1, 4], f32, tag="warmsig")
            nc.scalar.activation(warm_sig[0:1, 0:4], ssum4[0:1, 0:4], AF.Sigmoid)

            # combine chunks, normalize (0.25 head-mean folded into the
            # reciprocal), apply W_r per head, accumulate h2 in PSUM
            ssum2 = sb.tile([128, 2], f32, tag="ssum2")
            v2 = sb.tile([128, 2], f32, tag="v2")
            sview = ssum4[:].rearrange("p (b c) -> p b c", c=2)
            vview = vparts[:].rearrange("p (b c) -> p b c", c=2)
            nc.vector.tensor_tensor(ssum2[:], sview[:, :, 0], sview[:, :, 1],
                                    op=ALU.add)
            nc.vector.tensor_tensor(v2[:], vview[:, :, 0], vview[:, :, 1],
                                    op=ALU.add)
            rs2 = sb.tile([128, 2], f32, tag="rs2")
            nc.vector.reciprocal_approx_fast(rs2[:], ssum2[:])
            vn2 = sb.tile([128, 2], bf, tag="vn2")
            # vn = v * (1/sum) * 0.25 (head mean) in one fused DVE op
            nc.vector.scalar_tensor_tensor(out=vn2[:], in0=v2[:],
                                           scalar=1.0 / HEADS, in1=rs2[:],
                                           op0=ALU.mult, op1=ALU.mult)
            vnb = sb.tile([64, 2], bf, tag="vnb")
            nc.vector.tensor_copy(vnb[:], vn2[64:128, :])
            h2_ps = h2p_ps[:, 0:1]
            for h in range(RT):
                b, i = divmod(h, 2)
                rhs = vnb[:, b:b + 1] if i else vn2[0:64, b:b + 1]
                nc.tensor.matmul(h2_ps, B[0:64, C_WR + 64 * h:C_WR + 64 * h + 64],
                                 rhs, start=(h == 0), stop=(h == 3))
            h2b = sb.tile([128, 1], bf, tag="h2b")
            nc.vector.tensor_copy(h2b[64:128, :], h2_ps)

            # ---- final MLP (gated on h2) ----
            nc.tensor.matmul(c0_ps, B[64:128, C_WD0B:C_WD0B + H],
                             h2b[64:128, :], start=True, stop=True,
                             tile_position=(64, 0))
            c0col = sb.tile([64, 1], f32, tag="c0col")
            nc.vector.tensor_scalar_add(c0col[:], c0_ps, bd0c)
            y0b = sb.tile([128, SHARD], bf, tag="y0b")
            nc.scalar.activation(y0b[64:128, :], y0_ps, AF.Prelu, bias=c0col[:],
                                 alpha=SLOPE)
            nc.tensor.matmul(y1_ps, B[64:128, C_WD1:C_WD1 + SHARD],
                             y0b[64:128, :], start=True, stop=True,
                             tile_position=(64, 0))
            y1f = sb.tile([128, SHARD], bf, tag="y1f")
            nc.scalar.activation(y1f[:], y1_ps, AF.Prelu, bias=bd1c, alpha=SLOPE)
            nc.tensor.matmul(o_ps, B[:, C_WD2:C_WD2 + RT], y1f[:],
                             start=True, stop=True)
            o_sb = sb.tile([RT, SHARD], f32, tag="osb")
            nc.scalar.activation(o_sb[:], o_ps, AF.Sigmoid, bias=bd2c)
            nc.sync.dma_start(outT_d[:], o_sb[:])

    nc.compile()
    return nc


def _prep_inputs(inputs):
    import ml_dtypes
    bf16 = ml_dtypes.bfloat16
    f32 = np.float32

    hidden = np.asarray(inputs["hidden"], f32)
    ambiguous = np.asarray(inputs["ambiguous"], f32)
    type_agents = np.asarray(inputs["type_agents"], f32)
    W_self = np.asarray(inputs["W_self"], f32)
    b_self = np.asarray(inputs["b_self"], f32)
    W_merge = np.asarray(inputs["W_merge"], f32)
    b_merge = np.asarray(inputs["b_merge"], f32)
    W_trans = np.asarray(inputs["W_trans"], f32)
    b_trans = np.asarray(inputs["b_trans"], f32)
    W_l = np.asarray(inputs["W_l"], f32)
    W_r = np.asarray(inputs["W_r"], f32)
    w_attn = np.asarray(inputs["w_attn"], f32)
    Wd0 = np.asarray(inputs["Wd0"], f32)
    bd0 = np.asarray(inputs["bd0"], f32)
    Wd1 = np.asarray(inputs["Wd1"], f32)
    bd1 = np.asarray(inputs["bd1"], f32)
    Wd2 = np.asarray(inputs["Wd2"], f32)
    bd2 = np.asarray(inputs["bd2"], f32)

    base = np.zeros((128, CB), f32)
    top = base[0:64]
    bot = base[64:128]
    top[:, C_WL:C_WL + 256] = W_l.T
    top[:, C_WR:C_WR + 256] = W_r.T
    top[:, C_HT + 1:C_HT + N] = ambiguous.T
    bot[:, C_WT:C_WT + 256] = np.concatenate(
        [W_trans[t].T for t in range(RT)], axis=1) / APT
    bot[:, C_WSELF:C_WSELF + H] = W_self.T
    bot[:, C_WML:C_WML + H] = W_merge[:, :H].T
    bot[:, C_WMR:C_WMR + H] = W_merge[:, H:].T
    bot[:, C_WD0B:C_WD0B + H] = Wd0[:, H:].T
    bot[:, C_WD0A:C_WD0A + H] = Wd0[:, :H].T
    bot[:, C_WD1:C_WD1 + SHARD] = Wd1.T
    bot[:, C_TA:C_TA + RT * APT] = type_agents.reshape(RT * APT, H).T
    bot[:, C_BTT:C_BTT + RT] = b_trans.T
    bot[:, C_HID:C_HID + 1] = hidden.T
    bot[:, C_BSC] = b_self
    bot[:, C_BMC] = b_merge
    wexp = np.zeros((128, 128), f32)
    for hh in range(2):
        wexp[hh * 64:(hh + 1) * 64, hh * 64:(hh + 1) * 64] = w_attn[:, None]
    base[:, C_WEXP:C_WEXP + 128] = wexp
    base[:, C_WD2:C_WD2 + RT] = Wd2.T
    base[:, C_BD1] = bd1
    top[:, C_BD0] = bd0
    base[0:RT, C_BD2] = bd2

    amb_pad = np.zeros((H, NCORES * SHARD), f32)
    amb_pad[:, :N_AMB] = ambiguous.T
    in_maps = []
    for cidx in range(NCORES):
        blob = base.copy()
        blob[64:128, C_MLP:C_MLP + SHARD] = \
            amb_pad[:, cidx * SHARD:(cidx + 1) * SHARD]
        in_maps.append({"bfb": blob.astype(bf16)})
    return in_maps


def kernel(**inputs) -> np.ndarray:
    global _compiled
    if _compiled is None:
        _compiled = _build()
    nc = _compiled
    from concourse import bass_utils

    in_maps = _prep_inputs(inputs)
    res = bass_utils.run_bass_kernel_spmd(nc, in_maps, core_ids=list(range(NCORES)))
    out = np.empty((N_AMB, RT), np.float32)
    for cidx in range(NCORES):
        lo = cidx * SHARD
        hi = min(lo + SHARD, N_AMB)
        out[lo:hi, :] = res.results[cidx]["outT"][:, :hi - lo].T
    return out


# revision 19
# speedup vs baseline: 1.0354x; 1.0147x over previous
"""Trainium2 Bass kernel for nn_MlroleNode_64716567216639 (GAT message passing).

Math note: the reference computes a dense NxN GATv2 attention but only row 0
of the output feeds the final MLP, so this kernel computes just that row:
e[j,h] = leaky(g_l[j] + g_r[0]) . w_attn over the 1024 source nodes, softmax,
weighted sum of g_r values, then the 3-layer type-define MLP over the 1023
ambiguous nodes (sharded 128 nodes per core; GAT row-0 replicated).

Optimizations vs the naive version:
- All inputs packed into ONE bf16 blob + one tiny fp32 blob -> 3 dma_starts
  instead of 22 (each dma_start costs ~600ns serially on the Sync engine).
- 64-row weights ride the unused bottom partitions (64:128) of the blob; the
  matmuls that consume them run in the lower PE quadrant via
  tile_position=(64, .).
- bf16 matmuls: single PE pass (fp32 runs LOW_HIGH = 4 passes).
- leaky(x + bias) fused into one scalar-engine ACT (Prelu, alpha=0.2) reading
  straight from PSUM. Prelu lives in the same ACT table as Exp -> no table
  switches; Sigmoid's table is preloaded via a dummy ACT after the last Exp.
- softmax 1/sum via the single-op approximate reciprocal instead of the
  ~1.1us DVE reciprocal.
"""
import numpy as np

H = 64
N_AMB = 1023
N = 1024
HEADS = 4
RT = 4
APT = 3
SLOPE = 0.2
NCORES = 8
SHARD = 128

# bf16 blob column map (see _prep_inputs)
C_WL = 0        # top: W_l.T            [64, 256]
C_WR = 256      # top: W_r.T            [64, 256]
C_HT = 512      # top: hT (node 0 = h1 slot, zero), nodes j at col C_HT+j
C_WT = 0        # bottom: (W_mR @ W_trans[t]).T/3  [64, 256]
C_WSELF = 256   # bottom: W_self.T      [64, 64]
C_WML = 320     # bottom: W_merge[:, :64].T
C_WMR = 384     # bottom: W_merge[:, 64:].T
C_TA = 448      # bottom: type agents   [64, 12]
C_BTT = 460     # bottom: (W_mR @ b_trans[t] + b_merge).T  [64, 4]
C_HID = 464     # bottom: hidden.T      [64, 1]
C_BSC = 465     # bottom: b_self        [64, 1]
C_BMC = 466     # bottom: b_merge       [64, 1]
C_WD0B = 468    # bottom: Wd0[:, 64:].T
C_MLP = 532     # bottom: per-core mlp amb slice [64, 128]
C_WD0A = 660    # bottom: Wd0[:, :64].T
C_WD1 = 724     # bottom: Wd1.T         [64, 128]
C_WEXP = 1536   # full: block-diag w_attn  [128, 128]
C_WD2 = 1664    # full: Wd2.T           [128, 4]
C_BD1 = 1668    # full col: bd1
C_BD0 = 1669    # top rows 0:64: bd0
C_BD2 = 1670    # top rows 0:4: bd2
CB = 1671

_compiled = None


def _build():
    import concourse.tile as tile
    from concourse import bacc, mybir

    f32 = mybir.dt.float32
    bf = mybir.dt.bfloat16
    AF = mybir.ActivationFunctionType
    ALU = mybir.AluOpType
    AX = mybir.AxisListType

    nc = bacc.Bacc("TRN2", target_bir_lowering=False, debug=False,
                   enable_asserts=False, num_devices=NCORES)

    bfb_d = nc.dram_tensor("bfb", [128, CB], bf, kind="ExternalInput").ap()
    outT_d = nc.dram_tensor("outT", [RT, SHARD], f32, kind="ExternalOutput").ap()

    with nc.allow_low_precision("bf16 kernel, tolerance 2e-2"), \
         tile.TileContext(nc) as tc:
        with tc.tile_pool(name="wp", bufs=1) as wp, \
             tc.tile_pool(name="sb", bufs=1) as sb, \
             tc.tile_pool(name="ps", bufs=1, space="PSUM") as ps:

            B = wp.tile([128, CB], bf, tag="bfb")
            # ordered by when consumers need the data: prologue weights
            # first (the serial merge chain is the head of the critical
            # path), then biases, the full top half (W_l/W_r + hT), the
            # full-height tail (Wexp/fold/Wd2), and the MLP weights last.
            # Bottom cols 852:1536 are zeros and never transferred.
            nc.sync.dma_start(B[64:128, C_WSELF:C_WD0B], bfb_d[64:128, C_WSELF:C_WD0B])
            nc.sync.dma_start(B[64:128, 0:C_WSELF], bfb_d[64:128, 0:C_WSELF])
            nc.sync.dma_start(B[0:64, 0:C_WEXP], bfb_d[0:64, 0:C_WEXP])
            nc.sync.dma_start(B[:, C_WEXP:CB], bfb_d[:, C_WEXP:CB])
            nc.sync.dma_start(B[64:128, C_WD0B:852], bfb_d[64:128, C_WD0B:852])

            # biases travel as bf16 in the blob; DVE ops need fp32 scalar
            # operands, so widen them into small fp32 tiles right after the
            # carrying DMA lands (all off the critical path)
            biasP = sb.tile([128, 1], f32, tag="biasP")
            nc.vector.tensor_copy(biasP[64:128, :], B[64:128, C_BSC:C_BSC + 1])
            bsc = biasP[64:128, 0:1]
            biasM = sb.tile([128, 3], f32, tag="biasM")
            nc.vector.tensor_copy(biasM[:, 0:1], B[0:128, C_BD1:C_BD1 + 1])
            nc.vector.tensor_copy(biasM[0:64, 1:3], B[0:64, C_BD0:C_BD0 + 2])
            bd1c = biasM[0:128, 0:1]
            bd0c = biasM[0:64, 1:2]
            bd2c = biasM[0:4, 2:3]

            # preload the Exp table off the critical path (Prelu/Identity/Exp
            # all live in the same table set)
            warm = wp.tile([1, 4], f32, tag="warm")
            nc.vector.memset(warm[:], 0.0)
            warm_act = wp.tile([1, 4], f32, tag="warmact")
            nc.scalar.activation(warm_act[0:1, 0:1], warm[0:1, 0:1], AF.Exp)

            # PSUM arenas for small matmul outputs (bank-granular alloc).
            # Two separate banks: the tile-level dependency tracking adds
            # false write-after-read ordering between unrelated regions of
            # one tile, which serialized the prologue when shared.
            arenaM = ps.tile([128, 512], f32, tag="spM", bufs=1)
            C_ps = arenaM[64:128, 400:404]
            y0_ps = arenaM[0:64, 0:SHARD]
            y1_ps = arenaM[0:128, 128:256]
            o_ps = arenaM[0:4, 256:384]
            h2p_ps = arenaM[0:64, 384:386]
            c0_ps = arenaM[0:64, 388:389]
            h1_ps = ps.tile([128, 1], f32, tag="hp", bufs=2)

            # ---- prologue ----
            # h1 = W_self @ hidden + b_self goes first: it heads the serial
            # merge chain, and the C-path below runs concurrently with it
            h1t = sb.tile([128, RT + 1], bf, tag="h1t")
            nc.tensor.matmul(h1_ps[64:128, :], B[64:128, C_WSELF:C_WSELF + H],
                             B[64:128, C_HID:C_HID + 1], start=True, stop=True,
                             tile_position=(64, 64))
            nc.scalar.activation(h1t[64:128, 0:1], h1_ps[64:128, :], AF.Identity,
                                 bias=bsc)

            # role-type routing (the per-iteration bias columns C_sb)
            tsum = sb.tile([128, RT], bf, tag="tsum")
            nc.vector.reduce_sum(
                tsum[64:128, :],
                B[64:128, C_TA:C_TA + RT * APT].rearrange("p (t a) -> p t a", a=APT),
                axis=AX.X)
            # C_t = (W_mR W_trans[t]/3) @ tsum_t + (W_mR b_trans[t] + b_merge)
            # with both weight products precomputed on the host
            for t in range(RT):
                nc.tensor.matmul(C_ps[:, t:t + 1],
                                 B[64:128, C_WT + H * t:C_WT + H * (t + 1)],
                                 tsum[64:128, t:t + 1], start=True, stop=True,
                                 tile_position=(64, 64))
            C_sb = sb.tile([128, RT], f32, tag="C")
            nc.vector.tensor_tensor(C_sb[64:128, :], C_ps[:],
                                    B[64:128, C_BTT:C_BTT + RT], op=ALU.add)

            # 4x leaky-merge chain
            for t in range(RT):
                hp = ps.tile([128, 1], f32, tag="hp", bufs=2)
                nc.tensor.matmul(hp[64:128, :], B[64:128, C_WML:C_WML + H],
                                 h1t[64:128, t:t + 1], start=True, stop=True,
                                 tile_position=(64, 64))
                if t < RT - 1:
                    nc.scalar.activation(h1t[64:128, t + 1:t + 2], hp[64:128, :],
                                         AF.Prelu, bias=C_sb[64:128, t:t + 1],
                                         alpha=SLOPE)
                else:
                    # final h1 -> node-0 column of hT (top half)
                    nc.scalar.activation(B[0:64, C_HT:C_HT + 1], hp[64:128, :],
                                         AF.Prelu, bias=C_sb[64:128, t:t + 1],
                                         alpha=SLOPE)

            # attention query columns g_r[0] per head-pair block
            gr0c = sb.tile([128, 2], f32, tag="gr0c")
            gr0_ps = ps.tile([128, 2], f32, tag="gr0", bufs=1)
            for b in range(2):
                nc.tensor.matmul(gr0_ps[:, b:b + 1],
                                 B[0:64, C_WR + 128 * b:C_WR + 128 * b + 128],
                                 B[0:64, C_HT:C_HT + 1], start=True, stop=True)
            nc.vector.tensor_copy(gr0c[:], gr0_ps[:])

            # duplicated hT chunks: top = bottom = chunk, so one DVE pass
            # per unit can weight BOTH heads (pexp rows 0:64 and 64:128)
            # against the node features with all operands at base partition 0
            hdup = wp.tile([128, N], bf, tag="hdup")
            for c in range(2):
                cols = slice(C_HT + 512 * c, C_HT + 512 * (c + 1))
                nc.vector.tensor_copy(hdup[0:64, 512 * c:512 * (c + 1)], B[0:64, cols])
                nc.vector.tensor_copy(hdup[64:128, 512 * c:512 * (c + 1)], B[0:64, cols])


            # ---- first MLP matmul on this core's shard (h2-independent) ----
            nc.tensor.matmul(y0_ps, B[64:128, C_WD0A:C_WD0A + H],
                             B[64:128, C_MLP:C_MLP + SHARD], start=True, stop=True,
                             tile_position=(64, 0))

            # ---- GAT row 0: 2 head-pair blocks x 2 column chunks of 512.
            # Value aggregation uses linearity: sum_j a_j (W_r h_j) =
            # W_r (sum_j a_j h_j), so no big g_r matmuls are needed; the
            # weighted sums run on DVE straight against the bf16 hT columns
            # and W_r is applied once per head to a single 64-vector. ----
            ssum4 = sb.tile([128, 4], f32, tag="ssum4")
            vparts = sb.tile([128, 4], f32, tag="vparts")  # col = unit
            # pass 1: gl matmuls + fused leaky(gl + gr0) -> t_sb
            gl_list, t_list = [], []
            for b in range(2):
                for c in range(2):
                    cols = slice(C_HT + 512 * c, C_HT + 512 * (c + 1))
                    gl_ps = ps.tile([128, 512], f32, tag="ge", bufs=3)
                    nc.tensor.matmul(gl_ps[:],
                                     B[0:64, C_WL + 128 * b:C_WL + 128 * b + 128],
                                     B[0:64, cols], start=True, stop=True)
                    t_sb = sb.tile([128, 512], bf, tag="t", bufs=4)
                    nc.scalar.activation(t_sb[:], gl_ps[:], AF.Prelu,
                                         bias=gr0c[:, b:b + 1], alpha=SLOPE)
                    t_list.append(t_sb)
            # pass 2: attention logits -> exp -> per-head weighted node sums
            for b in range(2):
                for c in range(2):
                    u = 2 * b + c
                    cols = slice(C_HT + 512 * c, C_HT + 512 * (c + 1))
                    e_ps = ps.tile([128, 512], f32, tag="ge", bufs=3)
                    nc.tensor.matmul(e_ps[:], B[:, C_WEXP:C_WEXP + 128],
                                     t_list[u][:], start=True, stop=True)
                    pexp = sb.tile([128, 512], bf, tag="pexp", bufs=3)
                    nc.scalar.activation(pexp[:], e_ps[:], AF.Exp, bias=0.0,
                                         accum_out=ssum4[:, u:u + 1])
                    scr = sb.tile([128, 512], bf, tag="scr", bufs=3)
                    nc.vector.scalar_tensor_tensor(
                        out=scr[:], in0=pexp[:], scalar=1.0,
                        in1=hdup[:, 512 * c:512 * (c + 1)],
                        op0=ALU.mult, op1=ALU.mult,
                        accum_out=vparts[:, u:u + 1])

            # preload the Sigmoid table while the MLP matmuls run. Reading a
            # row of ssum4 makes this depend on ALL four Exp accumulators, so
            # the table switch is ordered strictly after the last Exp (the
            # same table also holds Prelu, so later Prelu ACTs don't reload).
            warm_sig = wp.tile([1, 4], f32, tag="warmsig")
            nc.scalar.activation(warm_sig[0:1, 0:4], ssum4[0:1, 0:4], AF.Sigmoid)

            # combine chunks, normalize (0.25 head-mean folded into the
            # reciprocal), apply W_r per head, accumulate h2 in PSUM
            ssum2 = sb.tile([128, 2], f32, tag="ssum2")
            v2 = sb.tile([128, 2], f32, tag="v2")
            sview = ssum4[:].rearrange("p (b c) -> p b c", c=2)
            vview = vparts[:].rearrange("p (b c) -> p b c", c=2)
            nc.vector.tensor_tensor(ssum2[:], sview[:, :, 0], sview[:, :, 1],
                                    op=ALU.add)
            nc.vector.tensor_tensor(v2[:], vview[:, :, 0], vview[:, :, 1],
                                    op=ALU.add)
            rs2 = sb.tile([128, 2], f32, tag="rs2")
            nc.vector.reciprocal_approx_fast(rs2[:], ssum2[:])
            vn2 = sb.tile([128, 2], bf, tag="vn2")
            # vn = v * (1/sum) * 0.25 (head mean) in one fused DVE op
            nc.vector.scalar_tensor_tensor(out=vn2[:], in0=v2[:],
                                           scalar=1.0 / HEADS, in1=rs2[:],
                                           op0=ALU.mult, op1=ALU.mult)
            vnb = sb.tile([64, 2], bf, tag="vnb")
            nc.vector.tensor_copy(vnb[:], vn2[64:128, :])
            h2_ps = h2p_ps[:, 0:1]
            for h in range(RT):
                b, i = divmod(h, 2)
                rhs = vnb[:, b:b + 1] if i else vn2[0:64, b:b + 1]
                nc.tensor.matmul(h2_ps, B[0:64, C_WR + 64 * h:C_WR + 64 * h + 64],
                                 rhs, start=(h == 0), stop=(h == 3))
            h2b = sb.tile([128, 1], bf, tag="h2b")
            nc.vector.tensor_copy(h2b[64:128, :], h2_ps)

            # ---- final MLP (gated on h2) ----
            nc.tensor.matmul(c0_ps, B[64:128, C_WD0B:C_WD0B + H],
                             h2b[64:128, :], start=True, stop=True,
                             tile_position=(64, 0))
            c0col = sb.tile([64, 1], f32, tag="c0col")
            nc.vector.tensor_scalar_add(c0col[:], c0_ps, bd0c)
            y0b = sb.tile([128, SHARD], bf, tag="y0b")
            nc.scalar.activation(y0b[64:128, :], y0_ps, AF.Prelu, bias=c0col[:],
                                 alpha=SLOPE)
            nc.tensor.matmul(y1_ps, B[64:128, C_WD1:C_WD1 + SHARD],
                             y0b[64:128, :], start=True, stop=True,
                             tile_position=(64, 0))
            y1f = sb.tile([128, SHARD], bf, tag="y1f")
            nc.scalar.activation(y1f[:], y1_ps, AF.Prelu, bias=bd1c, alpha=SLOPE)
            nc.tensor.matmul(o_ps, B[:, C_WD2:C_WD2 + RT], y1f[:],
                             start=True, stop=True)
            o_sb = sb.tile([RT, SHARD], f32, tag="osb")
            nc.scalar.activation(o_sb[:], o_ps, AF.Sigmoid, bias=bd2c)
            nc.sync.dma_start(outT_d[:], o_sb[:])

    nc.compile()
    return nc


def _prep_inputs(inputs):
    import ml_dtypes
    bf16 = ml_dtypes.bfloat16
    f32 = np.float32

    hidden = np.asarray(inputs["hidden"], f32)
    ambiguous = np.asarray(inputs["ambiguous"], f32)
    type_agents = np.asarray(inputs["type_agents"], f32)
    W_self = np.asarray(inputs["W_self"], f32)
    b_self = np.asarray(inputs["b_self"], f32)
    W_merge = np.asarray(inputs["W_merge"], f32)
    b_merge = np.asarray(inputs["b_merge"], f32)
    W_trans = np.asarray(inputs["W_trans"], f32)
    b_trans = np.asarray(inputs["b_trans"], f32)
    W_l = np.asarray(inputs["W_l"], f32)
    W_r = np.asarray(inputs["W_r"], f32)
    w_attn = np.asarray(inputs["w_attn"], f32)
    Wd0 = np.asarray(inputs["Wd0"], f32)
    bd0 = np.asarray(inputs["bd0"], f32)
    Wd1 = np.asarray(inputs["Wd1"], f32)
    bd1 = np.asarray(inputs["bd1"], f32)
    Wd2 = np.asarray(inputs["Wd2"], f32)
    bd2 = np.asarray(inputs["bd2"], f32)

    base = np.zeros((128, CB), f32)
    top = base[0:64]
    bot = base[64:128]
    top[:, C_WL:C_WL + 256] = W_l.T
    top[:, C_WR:C_WR + 256] = W_r.T
    top[:, C_HT + 1:C_HT + N] = ambiguous.T
    WmR = W_merge[:, H:]
    bot[:, C_WT:C_WT + 256] = np.concatenate(
        [(WmR @ W_trans[t]).T for t in range(RT)], axis=1) / APT
    bot[:, C_WSELF:C_WSELF + H] = W_self.T
    bot[:, C_WML:C_WML + H] = W_merge[:, :H].T
    bot[:, C_WMR:C_WMR + H] = W_merge[:, H:].T
    bot[:, C_WD0B:C_WD0B + H] = Wd0[:, H:].T
    bot[:, C_WD0A:C_WD0A + H] = Wd0[:, :H].T
    bot[:, C_WD1:C_WD1 + SHARD] = Wd1.T
    bot[:, C_TA:C_TA + RT * APT] = type_agents.reshape(RT * APT, H).T
    bot[:, C_BTT:C_BTT + RT] = (b_trans @ WmR.T + b_merge).T
    bot[:, C_HID:C_HID + 1] = hidden.T
    bot[:, C_BSC] = b_self
    wexp = np.zeros((128, 128), f32)
    for hh in range(2):
        wexp[hh * 64:(hh + 1) * 64, hh * 64:(hh + 1) * 64] = w_attn[:, None]
    base[:, C_WEXP:C_WEXP + 128] = wexp
    base[:, C_WD2:C_WD2 + RT] = Wd2.T
    base[:, C_BD1] = bd1
    top[:, C_BD0] = bd0
    base[0:RT, C_BD2] = bd2

    amb_pad = np.zeros((H, NCORES * SHARD), f32)
    amb_pad[:, :N_AMB] = ambiguous.T
    in_maps = []
    for cidx in range(NCORES):
        blob = base.copy()
        blob[64:128, C_MLP:C_MLP + SHARD] = \
            amb_pad[:, cidx * SHARD:(cidx + 1) * SHARD]
        in_maps.append({"bfb": blob.astype(bf16)})
    return in_maps


def kernel(**inputs) -> np.ndarray:
    global _compiled
    if _compiled is None:
        _compiled = _build()
    nc = _compiled
    from concourse import bass_utils

    in_maps = _prep_inputs(inputs)
    res = bass_utils.run_bass_kernel_spmd(nc, in_maps, core_ids=list(range(NCORES)))
    out = np.empty((N_AMB, RT), np.float32)
    for cidx in range(NCORES):
        lo = cidx * SHARD
        hi = min(lo + SHARD, N_AMB)
        out[lo:hi, :] = res.results[cidx]["outT"][:, :hi - lo].T
    return out
